# revision 46
# baseline (speedup 1.0000x reference)
"""3-layer GCN (EnhancedGraphNeuralNetwork) on 8 Trainium2 NeuronCores.

Strategy (dst-node sharded, graph-parallel):
  - Host: relabel nodes by descending degree, split 128-node blocks
    round-robin across 8 cores. Non-self edges bucketed by (dst core,
    dst block-group, 32k src window) and packed into 128-edge gather
    slots at GROUP granularity (a slot may straddle two dst blocks;
    each block gets its own "view" of that slot). Per-edge coefficient
    dinv_src*dinv_dst is folded into the one-hot aggregation matrices.
    Self-loops never go through the gather: each block has one "self"
    strip slot filled by a plain contiguous DMA from the core's own
    shard rows, with coefficient dinv^2 in its view column.
  - Device: layer 1 gathers raw x rows from a host-provided node-major
    table (no table build / AllGather on the critical path) and applies
    W1 after aggregation; layers 2/3 build table = act @ W per shard
    (matmul emits [node, F] rows directly — no transposes anywhere),
    AllGather bf16 tables, dma_gather source rows (4 SWDGE queues),
    then per dst block build all one-hot S matrices in 2 batched DVE
    ops (is_equal vs iota, * coef) and accumulate in PSUM:
       layers 1/2 (flipped):  Z[:, blk]  = sum_v strip_slot^T @ S_v
       layer 3:               Z3[blk, :] = sum_v S_v^T @ strip_slot
    Biases b1/b2 are dropped (BatchNorm cancels them exactly).
  - BatchNorm: per-block hardware bn_stats accumulated during the
    aggregation sweep + 1KB AllReduce; pad node columns are exactly
    zero so stats are exact.
"""

import math
import numpy as np
import ml_dtypes

import concourse.bass as bass
import concourse.bacc as bacc
import concourse.tile as tile
import concourse.mybir as mybir
from concourse.bass_utils import run_bass_kernel_spmd

N_CORES = 8
P = 128
EPS = 1e-5
WIN = 32768          # int16-addressable source window (table rows)
GROUP = 3            # dst blocks per gather group

FP = mybir.dt.float32
BF = mybir.dt.bfloat16
I16 = mybir.dt.int16

PADLOC = 1000.0      # dloc value for padding entries (kills one-hot row)


# ---------------------------------------------------------------- host prep

def _host_prep(x, edge_index, n_nodes):
    N = n_nodes
    NPAD = ((N + (P * N_CORES) - 1) // (P * N_CORES)) * (P * N_CORES)
    J = NPAD // P // N_CORES          # blocks per core
    SH = J * P                        # nodes per core shard
    NW = (NPAD + WIN - 1) // WIN
    NGRP = (J + GROUP - 1) // GROUP

    src = edge_index[0].astype(np.int64)
    dst = edge_index[1].astype(np.int64)

    # degree includes self loops
    deg = (np.bincount(np.concatenate([dst, np.arange(N)]), minlength=N)
           .astype(np.float64))
    dinv = 1.0 / np.sqrt(deg)
    coef = (dinv[src] * dinv[dst]).astype(np.float32)

    order = np.argsort(-deg, kind="stable")               # new id -> old id
    newid_of = np.empty(N, dtype=np.int64)
    newid_of[order] = np.arange(N)

    nsrc = newid_of[src]
    ndst = newid_of[dst]

    # table order: node n (new id) -> table row t(n). Shards are split in
    # four quarters so each quarter-table is one AllGather of
    # quarter-shards; gather window w == quarter w, so layer l+1's
    # window-w gathers can start as soon as AllGather q=w lands.
    SH4 = SH // 4
    NPAD4 = NPAD // 4
    g = np.arange(NPAD) // P
    c_of = g % N_CORES
    j_of = g // N_CORES
    q = j_of * P + (np.arange(NPAD) % P)      # position within shard
    quarter = q // SH4
    t_all = quarter * NPAD4 + c_of * SH4 + (q % SH4)

    # window w == quarter w
    assert NPAD4 <= WIN and NW == 4
    e_t = t_all[nsrc]                  # table row of source
    e_w = e_t // NPAD4
    e_rel = (e_t % NPAD4).astype(np.int32)
    e_g = ndst // P                    # dst global block
    e_c = (e_g % N_CORES).astype(np.int64)
    e_j = (e_g // N_CORES).astype(np.int64)
    e_p = (ndst % P).astype(np.float32)
    e_jg = e_j // GROUP
    e_jin = e_j % GROUP

    # sort by (core, group, window, block-in-group, src row)
    key = (((e_c * NGRP + e_jg) * NW + e_w) * GROUP + e_jin)
    o = np.lexsort((e_rel, key))
    rels, ps, cfs = e_rel[o], e_p[o], coef[o]
    bound = np.searchsorted(key[o], np.arange(N_CORES * NGRP * NW * GROUP + 1))

    def seg(c, jg, w, jin):
        k = ((c * NGRP + jg) * NW + w) * GROUP + jin
        return bound[k], bound[k + 1]

    # per-core per-node self coefficient (dinv^2), 0 for pad nodes
    selfco = np.zeros((N_CORES, SH), np.float32)
    shard_nids = np.empty((N_CORES, SH), np.int64)
    for c in range(N_CORES):
        gbs = np.arange(J) * N_CORES + c
        nids = (gbs[:, None] * P + np.arange(P)[None, :]).reshape(-1)
        shard_nids[c] = nids
        real = nids < N
        selfco[c][real] = (dinv[order[nids[real]]] ** 2).astype(np.float32)

    plan = []
    idx16 = [[] for _ in range(N_CORES)]   # per core [16, n/16] pieces
    dlv = [[] for _ in range(N_CORES)]     # per core [P] column pieces
    dvv = [[] for _ in range(N_CORES)]
    for jg in range(NGRP):
        blocks = list(range(jg * GROUP, min((jg + 1) * GROUP, J)))
        nbl = len(blocks)
        calls = []
        win_info = []
        kstart = 0
        o16_g = 0
        blk_wviews = {j: [] for j in blocks}
        for w in range(NW):
            lens = [[seg(c, jg, w, jin)[1] - seg(c, jg, w, jin)[0]
                     for jin in range(nbl)] for c in range(N_CORES)]
            Lj = [max(lens[c][bi] for c in range(N_CORES))
                  for bi in range(nbl)]
            Ltot = sum(Lj)
            if Ltot == 0:
                continue
            slots_w = (Ltot + P - 1) // P
            nidx = slots_w * P
            done = 0
            while done < nidx:
                piece = min(1024, nidx - done)
                calls.append((w, o16_g + done // 16, piece // 16,
                              kstart + done // P))
                done += piece
            offs = np.concatenate([[0], np.cumsum(Lj)])
            for bi, j in enumerate(blocks):
                if Lj[bi] > 0:
                    t0 = offs[bi] // P
                    t1 = (offs[bi] + Lj[bi] - 1) // P
                    for t in range(t0, t1 + 1):
                        blk_wviews[j].append((len(win_info), t))
            win_info.append((w, slots_w, kstart, offs))
            o16_g += nidx // 16
            kstart += slots_w

        core_vals = []
        for c in range(N_CORES):
            vparts = []
            wdata = []
            for (w, slots_w, ks, offs) in win_info:
                nidx = slots_w * P
                vals = np.zeros(nidx, np.int32)
                dvs = np.zeros(nidx, np.float32)
                pss = np.full(nidx, PADLOC, np.float32)
                apos = []
                for jin in range(nbl):
                    a, b = seg(c, jg, w, jin)
                    n = b - a
                    off = offs[jin]
                    vals[off:off + n] = rels[a:b]
                    dvs[off:off + n] = cfs[a:b]
                    pss[off:off + n] = ps[a:b]
                    apos.append((off, off + n))
                vparts.append(vals)
                wdata.append((pss, dvs, apos))
            allv = np.concatenate(vparts) if vparts else np.zeros(0, np.int32)
            wrapped = allv.reshape(-1, 16).T.astype(np.int16)
            idx16[c].append(np.tile(wrapped, (8, 1)))
            core_vals.append(wdata)

        # views ordered block-major; self view appended last per block
        views = {}
        selfslot = {}
        v_local = 0
        iota = np.arange(P, dtype=np.float32)
        for bi, j in enumerate(blocks):
            vlist = []
            for (wi, t) in blk_wviews[j]:
                w, slots_w, ks, offs = win_info[wi]
                for c in range(N_CORES):
                    pss, dvs, apos = core_vals[c][wi]
                    a, b = apos[bi]
                    col_l = np.full(P, PADLOC, np.float32)
                    col_v = np.zeros(P, np.float32)
                    lo, hi = max(a, t * P), min(b, (t + 1) * P)
                    if hi > lo:
                        col_l[lo - t * P:hi - t * P] = pss[lo:hi]
                        col_v[lo - t * P:hi - t * P] = dvs[lo:hi]
                    dlv[c].append(col_l)
                    dvv[c].append(col_v)
                vlist.append((v_local, ks + t))
                v_local += 1
            # self view
            sslot = kstart + bi
            for c in range(N_CORES):
                dlv[c].append(iota.copy())
                dvv[c].append(selfco[c][j * P:(j + 1) * P])
            vlist.append((v_local, sslot))
            v_local += 1
            views[j] = vlist
            selfslot[j] = sslot

        plan.append(dict(blocks=blocks, calls=calls, slots=kstart + nbl,
                         o16=o16_g, nv=v_local, views=views,
                         selfslot=selfslot))

    idx16 = np.stack([np.concatenate(idx16[c], axis=1)
                      for c in range(N_CORES)])
    dlv = np.stack([np.stack(dlv[c], axis=1) for c in range(N_CORES)])
    dvv = np.stack([np.stack(dvv[c], axis=1) for c in range(N_CORES)])

    # per-core shard features: transposed [F, SH] + node-major [SH, F],
    # and the full node-major raw-x table [NPAD, F] (layer-1 gather src)
    F_IN = x.shape[1]
    xsT = np.zeros((N_CORES, F_IN, SH), dtype=ml_dtypes.bfloat16)
    xsh = np.zeros((N_CORES, SH, F_IN), dtype=ml_dtypes.bfloat16)
    xtab = np.zeros((NPAD, F_IN), dtype=ml_dtypes.bfloat16)
    for c in range(N_CORES):
        nids = shard_nids[c]
        real = nids < N
        xr = np.zeros((SH, F_IN), dtype=np.float32)
        xr[real] = x[order[nids[real]]]
        xsh[c] = xr.astype(ml_dtypes.bfloat16)
        xsT[c] = np.ascontiguousarray(xr.T).astype(ml_dtypes.bfloat16)
        for qh in range(4):
            xtab[qh * NPAD4 + c * SH4:qh * NPAD4 + (c + 1) * SH4] = \
                xsh[c][qh * SH4:(qh + 1) * SH4]

    meta = dict(N=N, NPAD=NPAD, J=J, SH=SH, NW=NW, plan=plan,
                o16_total=idx16.shape[2], nv_total=dlv.shape[2],
                order=order)
    return meta, dict(xshT=xsT, xsh=xsh,
                      xtab=np.broadcast_to(
                          xtab, (N_CORES,) + xtab.shape).copy(),
                      idx16=idx16,
                      dlv=dlv.astype(ml_dtypes.bfloat16),
                      dvv=dvv.astype(ml_dtypes.bfloat16))


# ---------------------------------------------------------------- device

def _build(meta, hid, n_cls):
    J, SH = meta["J"], meta["SH"]
    N, NPAD = meta["N"], meta["NPAD"]
    plan = meta["plan"]
    F = hid
    O16, NV = meta["o16_total"], meta["nv_total"]
    SLOTS_MAX = max(pl["slots"] for pl in plan)
    O16_MAX = max(pl["o16"] for pl in plan)
    NV_MAX = max(pl["nv"] for pl in plan)
    NVB_MAX = max(len(v) for pl in plan for v in pl["views"].values())

    nc = bacc.Bacc("TRN2", target_bir_lowering=False, debug=False,
                   num_devices=N_CORES, num_swdge_queues=4)

    xshT_d = nc.dram_tensor("xshT", [F, SH], BF, kind="ExternalInput")
    xsh_d = nc.dram_tensor("xsh", [SH, F], BF, kind="ExternalInput")
    xtab_d = nc.dram_tensor("xtab", [NPAD, F], BF, kind="ExternalInput")
    idx_d = nc.dram_tensor("idx16", [P, O16], I16, kind="ExternalInput")
    dlv_d = nc.dram_tensor("dlv", [P, NV], BF, kind="ExternalInput")
    dvv_d = nc.dram_tensor("dvv", [P, NV], BF, kind="ExternalInput")
    iota_d = nc.dram_tensor("iotar", [P, P], BF, kind="ExternalInput")
    W1_d = nc.dram_tensor("W1", [F, F], FP, kind="ExternalInput")
    W2_d = nc.dram_tensor("W2", [F, F], FP, kind="ExternalInput")
    W3_d = nc.dram_tensor("W3", [F, n_cls], FP, kind="ExternalInput")
    b3_d = nc.dram_tensor("b3", [n_cls], FP, kind="ExternalInput")
    g1_d = nc.dram_tensor("g1", [F], FP, kind="ExternalInput")
    be1_d = nc.dram_tensor("be1", [F], FP, kind="ExternalInput")
    g2_d = nc.dram_tensor("g2", [F], FP, kind="ExternalInput")
    be2_d = nc.dram_tensor("be2", [F], FP, kind="ExternalInput")
    out_d = nc.dram_tensor("out", [P, J, n_cls], FP, kind="ExternalOutput")

    with tile.TileContext(nc) as tc:
        with (
            tc.tile_pool(name="persist", bufs=1) as pp,
            tc.tile_pool(name="blk", bufs=3) as bp,
            tc.tile_pool(name="spool", bufs=3) as sp,
            tc.tile_pool(name="meta", bufs=3) as mp,
            tc.tile_pool(name="gath", bufs=4) as gp,
            tc.tile_pool(name="psum", bufs=2, space="PSUM") as psp,
            tc.tile_pool(name="dram", bufs=1, space="DRAM") as dp,
        ):
            # ---------- constants
            iotar = pp.tile([P, P], BF, tag="iotar")
            nc.sync.dma_start(out=iotar[:], in_=iota_d[:])

            w1 = pp.tile([F, F], BF, tag="w1")
            w2 = pp.tile([F, F], BF, tag="w2")
            w3 = pp.tile([F, n_cls], BF, tag="w3")
            nc.gpsimd.dma_start(out=w1[:], in_=W1_d[:])
            nc.gpsimd.dma_start(out=w2[:], in_=W2_d[:])
            nc.gpsimd.dma_start(out=w3[:], in_=W3_d[:])

            def col(dram1d, n=F):
                t = pp.tile([n, 1], FP, tag=f"col_{dram1d.name}")
                nc.sync.dma_start(out=t[:], in_=dram1d[:, None])
                return t

            g1c, be1c, g2c, be2c = col(g1_d), col(be1_d), col(g2_d), col(be2_d)
            b3bc = pp.tile([P, n_cls], FP, tag="b3bc")
            nc.gpsimd.dma_start(
                out=b3bc[:],
                in_=bass.AP(tensor=b3_d, offset=0, ap=[[0, P], [1, n_cls]]))
            epsc = pp.tile([P, 1], FP, tag="eps")
            nc.vector.memset(epsc[:], EPS)

            # ---------- big persistent buffers
            Z = pp.tile([F, SH], FP, tag="z")              # pre-BN activations
            xTbf = pp.tile([F, SH], BF, tag="actbuf")      # residual source
            nc.sync.dma_start(out=xTbf[:], in_=xshT_d[:])
            A1 = pp.tile([F, SH], BF, tag="actbuf")
            A2 = pp.tile([F, SH], BF, tag="actbuf")
            Z3 = pp.tile([P, J, n_cls], FP, tag="actbuf")
            stats = [pp.tile([P, J, 6], FP, tag=f"bnstats{i}",
                             name=f"bnstats{i}")
                     for i in range(2)]

            # ---------- DRAM internals
            NPAD4 = NPAD // 4
            SH4 = SH // 4
            agin = dp.tile([SH, F], BF, tag="agin")
            tables = {li: [dp.tile([NPAD4, F], BF, tag=f"table{li}_{h}",
                                   name=f"table{li}_{h}",
                                   addr_space="Shared")
                           for h in range(4)]
                      for li in (1, 2)}
            st_in = dp.tile([P, 2], FP, tag="stin")
            st_outs = [dp.tile([P, 2], FP, tag=f"stout{i}",
                               name=f"stout{i}", addr_space="Shared")
                       for i in range(2)]

            def allgather_quarter(li, h):
                nc.gpsimd.collective_compute(
                    "AllGather", mybir.AluOpType.bypass,
                    replica_groups=[list(range(N_CORES))],
                    ins=[agin[h * SH4:(h + 1) * SH4, :]],
                    outs=[tables[li][h][:]])

            # last block whose agin rows complete quarter q
            ag_after = [((q + 1) * SH4 - 1) // P for q in range(4)]

            # ---------- table build: agin rows = (src^T @ W) per block
            def table_build(src_bf, w_sb, n_out, li):
                for j in range(J):
                    ps = psp.tile([P, F], FP, tag="ps_tb")
                    nc.tensor.matmul(ps[:, :n_out],
                                     lhsT=src_bf[:, j * P:(j + 1) * P],
                                     rhs=w_sb[:], start=True, stop=True)
                    tb = bp.tile([P, F], BF, tag="tb")
                    if n_out < F:
                        nc.vector.memset(tb[:], 0.0)
                        nc.vector.tensor_copy(out=tb[:, :n_out],
                                              in_=ps[:, :n_out])
                    else:
                        nc.vector.tensor_copy(out=tb[:], in_=ps[:])
                    nc.sync.dma_start(out=agin[j * P:(j + 1) * P, :],
                                      in_=tb[:])
                    for q in range(4):
                        if j == ag_after[q]:
                            allgather_quarter(li, q)

            # ---------- one aggregation sweep over the shard
            def layer_agg(li):
                selfsrc = xsh_d if li == 0 else agin
                o16_base = 0
                v_base = 0
                for pl in plan:
                    o16_len = pl["o16"]
                    nv = pl["nv"]
                    idx_sb = mp.tile([P, O16_MAX], I16, tag="idxsb")
                    nc.sync.dma_start(
                        out=idx_sb[:, :o16_len],
                        in_=idx_d[:, o16_base:o16_base + o16_len])
                    dlv_sb = mp.tile([P, NV_MAX], BF, tag="dlvsb")
                    nc.sync.dma_start(
                        out=dlv_sb[:, :nv],
                        in_=dlv_d[:, v_base:v_base + nv])
                    dvv_sb = mp.tile([P, NV_MAX], BF, tag="dvvsb")
                    nc.sync.dma_start(
                        out=dvv_sb[:, :nv],
                        in_=dvv_d[:, v_base:v_base + nv])
                    strip = gp.tile([P, SLOTS_MAX, F], BF, tag="strip")
                    for ci, (w, o16, n16, kstart) in enumerate(pl["calls"]):
                        nidx = n16 * 16
                        if li == 0:
                            src_ap = xtab_d[w * NPAD4:(w + 1) * NPAD4, :]
                        else:
                            src_ap = tables[li][w][:, :]
                        nc.gpsimd.dma_gather(
                            out_ap=strip[:, kstart:kstart + nidx // P, :],
                            in_ap=src_ap,
                            idxs_ap=idx_sb[:, o16:o16 + n16],
                            num_idxs=nidx, num_idxs_reg=nidx, elem_size=F,
                            queue_num=ci % 4)
                    for j in pl["blocks"]:
                        nc.scalar.dma_start(
                            out=strip[:, pl["selfslot"][j], :],
                            in_=selfsrc[j * P:(j + 1) * P, :])
                    for bi, j in enumerate(pl["blocks"]):
                        vs = pl["views"][j]
                        nv_j = len(vs)
                        v0 = vs[0][0]
                        S = sp.tile([P, NVB_MAX, P], BF, tag="S")
                        nc.vector.tensor_tensor(
                            out=S[:, :nv_j, :],
                            in0=dlv_sb[:, v0:v0 + nv_j].unsqueeze(2)
                                .to_broadcast([P, nv_j, P]),
                            in1=iotar[:].unsqueeze(1)
                                .to_broadcast([P, nv_j, P]),
                            op=mybir.AluOpType.is_equal)
                        nc.vector.tensor_tensor(
                            out=S[:, :nv_j, :],
                            in0=S[:, :nv_j, :],
                            in1=dvv_sb[:, v0:v0 + nv_j].unsqueeze(2)
                                .to_broadcast([P, nv_j, P]),
                            op=mybir.AluOpType.mult)
                        if li < 2:
                            pagg = psp.tile([P, P], FP, tag="ps_agg")
                            for i, (v, s) in enumerate(vs):
                                nc.tensor.matmul(
                                    pagg[:], lhsT=strip[:, s, :],
                                    rhs=S[:, v - v0, :],
                                    start=(i == 0), stop=(i == nv_j - 1))
                            if li == 0:
                                # raw-x aggregation: apply W1 post-agg
                                aggsb = bp.tile([P, P], BF, tag="aggsb")
                                nc.scalar.activation(
                                    out=aggsb[:], in_=pagg[:],
                                    func=mybir.ActivationFunctionType.Copy)
                                pz = psp.tile([P, P], FP, tag="ps_z")
                                nc.tensor.matmul(pz[:], lhsT=w1[:],
                                                 rhs=aggsb[:],
                                                 start=True, stop=True)
                                src_ps = pz
                            else:
                                src_ps = pagg
                            nc.scalar.activation(
                                out=Z[:, j * P:(j + 1) * P], in_=src_ps[:],
                                func=mybir.ActivationFunctionType.Copy)
                            nc.vector.bn_stats(
                                out=stats[li][:, j, :],
                                in_=Z[:, j * P:(j + 1) * P])
                        else:
                            pagg = psp.tile([P, n_cls], FP, tag="ps_agg3")
                            for i, (v, s) in enumerate(vs):
                                nc.tensor.matmul(
                                    pagg[:], lhsT=S[:, v - v0, :],
                                    rhs=strip[:, s, :n_cls],
                                    start=(i == 0), stop=(i == nv_j - 1))
                            nc.vector.tensor_add(out=Z3[:, j, :],
                                                 in0=pagg[:], in1=b3bc[:])
                    if li == 2:
                        # log_softmax + output store for this group
                        j0 = pl["blocks"][0]
                        nbl = len(pl["blocks"])
                        zg = Z3[:, j0:j0 + nbl, :]
                        mx = bp.tile([P, GROUP, 1], FP, tag="mx")
                        nc.vector.reduce_max(out=mx[:, :nbl, :], in_=zg,
                                             axis=mybir.AxisListType.X)
                        nc.vector.tensor_sub(
                            out=zg, in0=zg,
                            in1=mx[:, :nbl, :].to_broadcast([P, nbl, n_cls]))
                        exg = bp.tile([P, GROUP, n_cls], FP, tag="exg")
                        nc.scalar.activation(
                            out=exg[:, :nbl, :], in_=zg,
                            func=mybir.ActivationFunctionType.Exp)
                        smg = bp.tile([P, GROUP, 1], FP, tag="smg")
                        nc.vector.reduce_sum(out=smg[:, :nbl, :],
                                             in_=exg[:, :nbl, :],
                                             axis=mybir.AxisListType.X)
                        lsg = bp.tile([P, GROUP, 1], FP, tag="lsg")
                        nc.scalar.activation(
                            out=lsg[:, :nbl, :], in_=smg[:, :nbl, :],
                            func=mybir.ActivationFunctionType.Ln)
                        nc.vector.tensor_sub(
                            out=zg, in0=zg,
                            in1=lsg[:, :nbl, :].to_broadcast([P, nbl, n_cls]))
                        nc.sync.dma_start(out=out_d[:, j0:j0 + nbl, :],
                                          in_=zg)
                    o16_base += o16_len
                    v_base += nv

            # ---------- BN (global) + act
            def bn_relu(g_col, be_col, residual, li, act_out):
                st_out = st_outs[li]
                mv = bp.tile([P, 2], FP, tag="bnmv")
                nc.vector.bn_aggr(out=mv[:], in_=stats[li][:])
                sums = bp.tile([P, 2], FP, tag="sums")
                musq = bp.tile([P, 1], FP, tag="musq")
                nc.vector.tensor_mul(out=musq[:], in0=mv[:, 0:1], in1=mv[:, 0:1])
                nc.scalar.mul(out=sums[:, 0:1], in_=mv[:, 0:1], mul=float(SH))
                nc.vector.tensor_add(out=sums[:, 1:2], in0=mv[:, 1:2],
                                     in1=musq[:])
                nc.scalar.mul(out=sums[:, 1:2], in_=sums[:, 1:2], mul=float(SH))
                nc.sync.dma_start(out=st_in[:], in_=sums[:])
                nc.gpsimd.collective_compute(
                    "AllReduce", mybir.AluOpType.add,
                    replica_groups=[list(range(N_CORES))],
                    ins=[st_in[:]], outs=[st_out[:]])
                gl = bp.tile([P, 2], FP, tag="gl")
                nc.sync.dma_start(out=gl[:], in_=st_out[:])
                mu = bp.tile([P, 1], FP, tag="mu")
                var = bp.tile([P, 1], FP, tag="var")
                nc.scalar.mul(out=mu[:], in_=gl[:, 0:1], mul=1.0 / N)
                nc.scalar.mul(out=var[:], in_=gl[:, 1:2], mul=1.0 / N)
                nc.vector.tensor_mul(out=musq[:], in0=mu[:], in1=mu[:])
                nc.vector.tensor_sub(out=var[:], in0=var[:], in1=musq[:])
                rstd = bp.tile([P, 1], FP, tag="rstd")
                nc.scalar.activation(out=rstd[:], in_=var[:],
                                     func=mybir.ActivationFunctionType.Sqrt,
                                     bias=epsc[:], scale=1.0)
                nc.vector.reciprocal(out=rstd[:], in_=rstd[:])
                sc = bp.tile([P, 1], FP, tag="sc")
                sh = bp.tile([P, 1], FP, tag="sh")
                nc.vector.tensor_mul(out=sc[:], in0=g_col[:], in1=rstd[:])
                nc.vector.tensor_mul(out=sh[:], in0=mu[:], in1=sc[:])
                nc.vector.tensor_sub(out=sh[:], in0=be_col[:], in1=sh[:])
                nc.vector.tensor_scalar(out=Z[:], in0=Z[:], scalar1=sc[:],
                                        scalar2=sh[:],
                                        op0=mybir.AluOpType.mult,
                                        op1=mybir.AluOpType.add)
                if residual:
                    nc.vector.tensor_add(out=Z[:], in0=Z[:], in1=xTbf[:])
                nc.scalar.activation(out=act_out[:], in_=Z[:],
                                     func=mybir.ActivationFunctionType.Relu)

            # ================= pipeline
            layer_agg(0)
            bn_relu(g1c, be1c, residual=True, li=0, act_out=A1)

            table_build(A1, w2, F, 1)
            layer_agg(1)
            bn_relu(g2c, be2c, residual=False, li=1, act_out=A2)

            table_build(A2, w3, n_cls, 2)
            layer_agg(2)          # includes per-group log_softmax + store

    nc.compile()
    return nc


def _make_in_maps(meta, arrs, inputs):
    iota_rows = np.tile(np.arange(P, dtype=np.float32)[None, :], (P, 1))
    shared = dict(
        iotar=iota_rows.astype(ml_dtypes.bfloat16),
        **{k: np.asarray(inputs[k], np.float32)
           for k in ("W1", "W2", "W3", "b3", "g1", "be1", "g2", "be2")})
    return [dict(xshT=arrs["xshT"][c], xsh=arrs["xsh"][c],
                 xtab=arrs["xtab"][c], idx16=arrs["idx16"][c],
                 dlv=arrs["dlv"][c], dvv=arrs["dvv"][c],
                 **shared) for c in range(N_CORES)]


def _unshard(meta, results, n_cls):
    J, SH = meta["J"], meta["SH"]
    out = np.empty((meta["NPAD"], n_cls), np.float32)
    for c in range(N_CORES):
        o = results[c]["out"]                             # [P, J, C]
        nids = ((np.arange(J) * N_CORES + c)[:, None] * P
                + np.arange(P)[None, :])
        out[nids.reshape(-1)] = o.transpose(1, 0, 2).reshape(SH, n_cls)
    full = np.empty((meta["N"], n_cls), np.float32)
    full[meta["order"]] = out[:meta["N"]]
    return full


# ---------------------------------------------------------------- entry

def kernel(x, edge_index, W1, b1, g1, be1, W2, b2, g2, be2, W3, b3):
    x = np.asarray(x, dtype=np.float32)
    edge_index = np.asarray(edge_index)
    N, F = x.shape
    C = np.asarray(W3).shape[1]

    meta, arrs = _host_prep(x, edge_index, N)
    nc = _build(meta, F, C)
    in_maps = _make_in_maps(meta, arrs, dict(
        W1=W1, W2=W2, W3=W3, b3=b3, g1=g1, be1=be1, g2=g2, be2=be2))
    res = run_bass_kernel_spmd(nc, in_maps, core_ids=list(range(N_CORES)))
    return _unshard(meta, res.results, C)


# revision 47
# speedup vs baseline: 1.0836x; 1.0836x over previous
"""3-layer GCN (EnhancedGraphNeuralNetwork) on 8 Trainium2 NeuronCores.

Strategy (dst-node sharded, graph-parallel):
  - Host: relabel nodes by descending degree, split 128-node blocks
    round-robin across 8 cores. Non-self edges bucketed by (dst core,
    dst block-group, 32k src window) and packed into 128-edge gather
    slots at GROUP granularity (a slot may straddle two dst blocks;
    each block gets its own "view" of that slot). Per-edge coefficient
    dinv_src*dinv_dst is folded into the one-hot aggregation matrices.
    Self-loops never go through the gather: each block has one "self"
    strip slot filled by a plain contiguous DMA from the core's own
    shard rows, with coefficient dinv^2 in its view column.
  - Device: layer 1 gathers raw x rows from a host-provided node-major
    table (no table build / AllGather on the critical path) and applies
    W1 after aggregation; layers 2/3 build table = act @ W per shard
    (matmul emits [node, F] rows directly — no transposes anywhere),
    AllGather bf16 tables, dma_gather source rows (4 SWDGE queues),
    then per dst block build all one-hot S matrices in 2 batched DVE
    ops (is_equal vs iota, * coef) and accumulate in PSUM:
       layers 1/2 (flipped):  Z[:, blk]  = sum_v strip_slot^T @ S_v
       layer 3:               Z3[blk, :] = sum_v S_v^T @ strip_slot
    Biases b1/b2 are dropped (BatchNorm cancels them exactly).
  - BatchNorm: per-block hardware bn_stats accumulated during the
    aggregation sweep + 1KB AllReduce; pad node columns are exactly
    zero so stats are exact.
"""

import math
import numpy as np
import ml_dtypes

import concourse.bass as bass
import concourse.bacc as bacc
import concourse.tile as tile
import concourse.mybir as mybir
from concourse.bass_utils import run_bass_kernel_spmd

N_CORES = 8
P = 128
EPS = 1e-5
WIN = 32768          # int16-addressable source window (table rows)
GROUP = 4            # dst blocks per gather group

FP = mybir.dt.float32
BF = mybir.dt.bfloat16
I16 = mybir.dt.int16

PADLOC = 1000.0      # dloc value for padding entries (kills one-hot row)


# ---------------------------------------------------------------- host prep

def _host_prep(x, edge_index, n_nodes):
    N = n_nodes
    NPAD = ((N + (P * N_CORES) - 1) // (P * N_CORES)) * (P * N_CORES)
    J = NPAD // P // N_CORES          # blocks per core
    SH = J * P                        # nodes per core shard
    NW = (NPAD + WIN - 1) // WIN
    NGRP = (J + GROUP - 1) // GROUP

    src = edge_index[0].astype(np.int64)
    dst = edge_index[1].astype(np.int64)

    # degree includes self loops
    deg = (np.bincount(np.concatenate([dst, np.arange(N)]), minlength=N)
           .astype(np.float64))
    dinv = 1.0 / np.sqrt(deg)
    coef = (dinv[src] * dinv[dst]).astype(np.float32)

    order = np.argsort(-deg, kind="stable")               # new id -> old id
    newid_of = np.empty(N, dtype=np.int64)
    newid_of[order] = np.arange(N)

    nsrc = newid_of[src]
    ndst = newid_of[dst]

    # table order: node n (new id) -> table row t(n). Shards are split in
    # four quarters so each quarter-table is one AllGather of
    # quarter-shards; gather window w == quarter w, so layer l+1's
    # window-w gathers can start as soon as AllGather q=w lands.
    SH4 = SH // 4
    NPAD4 = NPAD // 4
    g = np.arange(NPAD) // P
    c_of = g % N_CORES
    j_of = g // N_CORES
    q = j_of * P + (np.arange(NPAD) % P)      # position within shard
    quarter = q // SH4
    t_all = quarter * NPAD4 + c_of * SH4 + (q % SH4)

    # window w == quarter w
    assert NPAD4 <= WIN and NW == 4
    e_t = t_all[nsrc]                  # table row of source
    e_w = e_t // NPAD4
    e_rel = (e_t % NPAD4).astype(np.int32)
    e_g = ndst // P                    # dst global block
    e_c = (e_g % N_CORES).astype(np.int64)
    e_j = (e_g // N_CORES).astype(np.int64)
    e_p = (ndst % P).astype(np.float32)
    e_jg = e_j // GROUP
    e_jin = e_j % GROUP

    # sort by (core, group, window, block-in-group, src row)
    key = (((e_c * NGRP + e_jg) * NW + e_w) * GROUP + e_jin)
    o = np.lexsort((e_rel, key))
    rels, ps, cfs = e_rel[o], e_p[o], coef[o]
    bound = np.searchsorted(key[o], np.arange(N_CORES * NGRP * NW * GROUP + 1))

    def seg(c, jg, w, jin):
        k = ((c * NGRP + jg) * NW + w) * GROUP + jin
        return bound[k], bound[k + 1]

    # per-core per-node self coefficient (dinv^2), 0 for pad nodes
    selfco = np.zeros((N_CORES, SH), np.float32)
    shard_nids = np.empty((N_CORES, SH), np.int64)
    for c in range(N_CORES):
        gbs = np.arange(J) * N_CORES + c
        nids = (gbs[:, None] * P + np.arange(P)[None, :]).reshape(-1)
        shard_nids[c] = nids
        real = nids < N
        selfco[c][real] = (dinv[order[nids[real]]] ** 2).astype(np.float32)

    plan = []
    idx16 = [[] for _ in range(N_CORES)]   # per core [16, n/16] pieces
    dlv = [[] for _ in range(N_CORES)]     # per core [P] column pieces
    dvv = [[] for _ in range(N_CORES)]
    for jg in range(NGRP):
        blocks = list(range(jg * GROUP, min((jg + 1) * GROUP, J)))
        nbl = len(blocks)
        calls = []
        win_info = []
        kstart = 0
        o16_g = 0
        blk_wviews = {j: [] for j in blocks}
        for w in range(NW):
            lens = [[seg(c, jg, w, jin)[1] - seg(c, jg, w, jin)[0]
                     for jin in range(nbl)] for c in range(N_CORES)]
            Lj = [max(lens[c][bi] for c in range(N_CORES))
                  for bi in range(nbl)]
            Ltot = sum(Lj)
            if Ltot == 0:
                continue
            slots_w = (Ltot + P - 1) // P
            nidx = slots_w * P
            done = 0
            while done < nidx:
                piece = min(1024, nidx - done)
                calls.append((w, o16_g + done // 16, piece // 16,
                              kstart + done // P))
                done += piece
            offs = np.concatenate([[0], np.cumsum(Lj)])
            for bi, j in enumerate(blocks):
                if Lj[bi] > 0:
                    t0 = offs[bi] // P
                    t1 = (offs[bi] + Lj[bi] - 1) // P
                    for t in range(t0, t1 + 1):
                        blk_wviews[j].append((len(win_info), t))
            win_info.append((w, slots_w, kstart, offs))
            o16_g += nidx // 16
            kstart += slots_w

        core_vals = []
        for c in range(N_CORES):
            vparts = []
            wdata = []
            for (w, slots_w, ks, offs) in win_info:
                nidx = slots_w * P
                vals = np.zeros(nidx, np.int32)
                dvs = np.zeros(nidx, np.float32)
                pss = np.full(nidx, PADLOC, np.float32)
                apos = []
                for jin in range(nbl):
                    a, b = seg(c, jg, w, jin)
                    n = b - a
                    off = offs[jin]
                    vals[off:off + n] = rels[a:b]
                    dvs[off:off + n] = cfs[a:b]
                    pss[off:off + n] = ps[a:b]
                    apos.append((off, off + n))
                vparts.append(vals)
                wdata.append((pss, dvs, apos))
            allv = np.concatenate(vparts) if vparts else np.zeros(0, np.int32)
            wrapped = allv.reshape(-1, 16).T.astype(np.int16)
            idx16[c].append(np.tile(wrapped, (8, 1)))
            core_vals.append(wdata)

        # views ordered block-major; self view appended last per block
        views = {}
        selfslot = {}
        v_local = 0
        iota = np.arange(P, dtype=np.float32)
        for bi, j in enumerate(blocks):
            vlist = []
            for (wi, t) in blk_wviews[j]:
                w, slots_w, ks, offs = win_info[wi]
                for c in range(N_CORES):
                    pss, dvs, apos = core_vals[c][wi]
                    a, b = apos[bi]
                    col_l = np.full(P, PADLOC, np.float32)
                    col_v = np.zeros(P, np.float32)
                    lo, hi = max(a, t * P), min(b, (t + 1) * P)
                    if hi > lo:
                        col_l[lo - t * P:hi - t * P] = pss[lo:hi]
                        col_v[lo - t * P:hi - t * P] = dvs[lo:hi]
                    dlv[c].append(col_l)
                    dvv[c].append(col_v)
                vlist.append((v_local, ks + t))
                v_local += 1
            # self view
            sslot = kstart + bi
            for c in range(N_CORES):
                dlv[c].append(iota.copy())
                dvv[c].append(selfco[c][j * P:(j + 1) * P])
            vlist.append((v_local, sslot))
            v_local += 1
            views[j] = vlist
            selfslot[j] = sslot

        plan.append(dict(blocks=blocks, calls=calls, slots=kstart + nbl,
                         o16=o16_g, nv=v_local, views=views,
                         selfslot=selfslot))

    idx16 = np.stack([np.concatenate(idx16[c], axis=1)
                      for c in range(N_CORES)])
    dlv = np.stack([np.stack(dlv[c], axis=1) for c in range(N_CORES)])
    dvv = np.stack([np.stack(dvv[c], axis=1) for c in range(N_CORES)])

    # per-core shard features: transposed [F, SH] + node-major [SH, F],
    # and the full node-major raw-x table [NPAD, F] (layer-1 gather src)
    F_IN = x.shape[1]
    xsT = np.zeros((N_CORES, F_IN, SH), dtype=ml_dtypes.bfloat16)
    xsh = np.zeros((N_CORES, SH, F_IN), dtype=ml_dtypes.bfloat16)
    xtab = np.zeros((NPAD, F_IN), dtype=ml_dtypes.bfloat16)
    for c in range(N_CORES):
        nids = shard_nids[c]
        real = nids < N
        xr = np.zeros((SH, F_IN), dtype=np.float32)
        xr[real] = x[order[nids[real]]]
        xsh[c] = xr.astype(ml_dtypes.bfloat16)
        xsT[c] = np.ascontiguousarray(xr.T).astype(ml_dtypes.bfloat16)
        for qh in range(4):
            xtab[qh * NPAD4 + c * SH4:qh * NPAD4 + (c + 1) * SH4] = \
                xsh[c][qh * SH4:(qh + 1) * SH4]

    meta = dict(N=N, NPAD=NPAD, J=J, SH=SH, NW=NW, plan=plan,
                o16_total=idx16.shape[2], nv_total=dlv.shape[2],
                order=order)
    return meta, dict(xshT=xsT, xsh=xsh,
                      xtab=np.broadcast_to(
                          xtab, (N_CORES,) + xtab.shape).copy(),
                      idx16=idx16,
                      dlv=dlv.astype(ml_dtypes.bfloat16),
                      dvv=dvv.astype(ml_dtypes.bfloat16))


# ---------------------------------------------------------------- device

def _build(meta, hid, n_cls):
    J, SH = meta["J"], meta["SH"]
    N, NPAD = meta["N"], meta["NPAD"]
    plan = meta["plan"]
    F = hid
    O16, NV = meta["o16_total"], meta["nv_total"]
    SLOTS_MAX = max(pl["slots"] for pl in plan)
    O16_MAX = max(pl["o16"] for pl in plan)
    NV_MAX = max(pl["nv"] for pl in plan)
    NVB_MAX = max(len(v) for pl in plan for v in pl["views"].values())

    nc = bacc.Bacc("TRN2", target_bir_lowering=False, debug=False,
                   num_devices=N_CORES, num_swdge_queues=4)

    xshT_d = nc.dram_tensor("xshT", [F, SH], BF, kind="ExternalInput")
    xsh_d = nc.dram_tensor("xsh", [SH, F], BF, kind="ExternalInput")
    xtab_d = nc.dram_tensor("xtab", [NPAD, F], BF, kind="ExternalInput")
    idx_d = nc.dram_tensor("idx16", [P, O16], I16, kind="ExternalInput")
    dlv_d = nc.dram_tensor("dlv", [P, NV], BF, kind="ExternalInput")
    dvv_d = nc.dram_tensor("dvv", [P, NV], BF, kind="ExternalInput")
    iota_d = nc.dram_tensor("iotar", [P, P], BF, kind="ExternalInput")
    W1_d = nc.dram_tensor("W1", [F, F], FP, kind="ExternalInput")
    W2_d = nc.dram_tensor("W2", [F, F], FP, kind="ExternalInput")
    W3_d = nc.dram_tensor("W3", [F, n_cls], FP, kind="ExternalInput")
    b3_d = nc.dram_tensor("b3", [n_cls], FP, kind="ExternalInput")
    g1_d = nc.dram_tensor("g1", [F], FP, kind="ExternalInput")
    be1_d = nc.dram_tensor("be1", [F], FP, kind="ExternalInput")
    g2_d = nc.dram_tensor("g2", [F], FP, kind="ExternalInput")
    be2_d = nc.dram_tensor("be2", [F], FP, kind="ExternalInput")
    out_d = nc.dram_tensor("out", [P, J, n_cls], FP, kind="ExternalOutput")

    with tile.TileContext(nc) as tc:
        with (
            tc.tile_pool(name="persist", bufs=1) as pp,
            tc.tile_pool(name="blk", bufs=3) as bp,
            tc.tile_pool(name="spool", bufs=4) as sp,
            tc.tile_pool(name="meta", bufs=3) as mp,
            tc.tile_pool(name="gath", bufs=3) as gp,
            tc.tile_pool(name="psum", bufs=2, space="PSUM") as psp,
            tc.tile_pool(name="dram", bufs=1, space="DRAM") as dp,
        ):
            # ---------- constants
            iotar = pp.tile([P, P], BF, tag="iotar")
            nc.sync.dma_start(out=iotar[:], in_=iota_d[:])

            w1 = pp.tile([F, F], BF, tag="w1")
            w2 = pp.tile([F, F], BF, tag="w2")
            w3 = pp.tile([F, n_cls], BF, tag="w3")
            nc.gpsimd.dma_start(out=w1[:], in_=W1_d[:])
            nc.gpsimd.dma_start(out=w2[:], in_=W2_d[:])
            nc.gpsimd.dma_start(out=w3[:], in_=W3_d[:])

            def col(dram1d, n=F):
                t = pp.tile([n, 1], FP, tag=f"col_{dram1d.name}")
                nc.sync.dma_start(out=t[:], in_=dram1d[:, None])
                return t

            g1c, be1c, g2c, be2c = col(g1_d), col(be1_d), col(g2_d), col(be2_d)
            b3bc = pp.tile([P, n_cls], FP, tag="b3bc")
            nc.gpsimd.dma_start(
                out=b3bc[:],
                in_=bass.AP(tensor=b3_d, offset=0, ap=[[0, P], [1, n_cls]]))
            epsc = pp.tile([P, 1], FP, tag="eps")
            nc.vector.memset(epsc[:], EPS)

            # ---------- big persistent buffers
            Z = pp.tile([F, SH], FP, tag="z")              # pre-BN activations
            xTbf = pp.tile([F, SH], BF, tag="actbuf")      # residual source
            nc.sync.dma_start(out=xTbf[:], in_=xshT_d[:])
            A1 = pp.tile([F, SH], BF, tag="actbuf")
            A2 = pp.tile([F, SH], BF, tag="actbuf")
            Z3 = pp.tile([P, J, n_cls], FP, tag="actbuf")
            stats = [pp.tile([P, J, 6], FP, tag=f"bnstats{i}",
                             name=f"bnstats{i}")
                     for i in range(2)]

            # ---------- DRAM internals
            NPAD4 = NPAD // 4
            SH4 = SH // 4
            agin = dp.tile([SH, F], BF, tag="agin")
            tables = {li: [dp.tile([NPAD4, F], BF, tag=f"table{li}_{h}",
                                   name=f"table{li}_{h}",
                                   addr_space="Shared")
                           for h in range(4)]
                      for li in (1, 2)}
            st_in = dp.tile([P, 2], FP, tag="stin")
            st_outs = [dp.tile([P, 2], FP, tag=f"stout{i}",
                               name=f"stout{i}", addr_space="Shared")
                       for i in range(2)]

            def allgather_quarter(li, h):
                nc.gpsimd.collective_compute(
                    "AllGather", mybir.AluOpType.bypass,
                    replica_groups=[list(range(N_CORES))],
                    ins=[agin[h * SH4:(h + 1) * SH4, :]],
                    outs=[tables[li][h][:]])

            # last block whose agin rows complete quarter q
            ag_after = [((q + 1) * SH4 - 1) // P for q in range(4)]

            # ---------- table build: agin rows = (src^T @ W) per block
            def table_build(src_bf, w_sb, n_out, li):
                for j in range(J):
                    ps = psp.tile([P, F], FP, tag="ps_tb")
                    nc.tensor.matmul(ps[:, :n_out],
                                     lhsT=src_bf[:, j * P:(j + 1) * P],
                                     rhs=w_sb[:], start=True, stop=True)
                    tb = bp.tile([P, F], BF, tag="tb")
                    if n_out < F:
                        nc.vector.memset(tb[:], 0.0)
                        nc.vector.tensor_copy(out=tb[:, :n_out],
                                              in_=ps[:, :n_out])
                    else:
                        nc.vector.tensor_copy(out=tb[:], in_=ps[:])
                    nc.sync.dma_start(out=agin[j * P:(j + 1) * P, :],
                                      in_=tb[:])
                    for q in range(4):
                        if j == ag_after[q]:
                            allgather_quarter(li, q)

            # ---------- one aggregation sweep over the shard
            def layer_agg(li):
                selfsrc = xsh_d if li == 0 else agin
                o16_base = 0
                v_base = 0
                for pl in plan:
                    o16_len = pl["o16"]
                    nv = pl["nv"]
                    idx_sb = mp.tile([P, O16_MAX], I16, tag="idxsb")
                    nc.sync.dma_start(
                        out=idx_sb[:, :o16_len],
                        in_=idx_d[:, o16_base:o16_base + o16_len])
                    dlv_sb = mp.tile([P, NV_MAX], BF, tag="dlvsb")
                    nc.sync.dma_start(
                        out=dlv_sb[:, :nv],
                        in_=dlv_d[:, v_base:v_base + nv])
                    dvv_sb = mp.tile([P, NV_MAX], BF, tag="dvvsb")
                    nc.sync.dma_start(
                        out=dvv_sb[:, :nv],
                        in_=dvv_d[:, v_base:v_base + nv])
                    strip = gp.tile([P, SLOTS_MAX, F], BF, tag="strip")
                    for ci, (w, o16, n16, kstart) in enumerate(pl["calls"]):
                        nidx = n16 * 16
                        if li == 0:
                            src_ap = xtab_d[w * NPAD4:(w + 1) * NPAD4, :]
                        else:
                            src_ap = tables[li][w][:, :]
                        nc.gpsimd.dma_gather(
                            out_ap=strip[:, kstart:kstart + nidx // P, :],
                            in_ap=src_ap,
                            idxs_ap=idx_sb[:, o16:o16 + n16],
                            num_idxs=nidx, num_idxs_reg=nidx, elem_size=F,
                            queue_num=ci % 4)
                    for j in pl["blocks"]:
                        nc.scalar.dma_start(
                            out=strip[:, pl["selfslot"][j], :],
                            in_=selfsrc[j * P:(j + 1) * P, :])
                    for bi, j in enumerate(pl["blocks"]):
                        vs = pl["views"][j]
                        nv_j = len(vs)
                        v0 = vs[0][0]
                        S = sp.tile([P, NVB_MAX, P], BF, tag="S")
                        nc.vector.tensor_tensor(
                            out=S[:, :nv_j, :],
                            in0=dlv_sb[:, v0:v0 + nv_j].unsqueeze(2)
                                .to_broadcast([P, nv_j, P]),
                            in1=iotar[:].unsqueeze(1)
                                .to_broadcast([P, nv_j, P]),
                            op=mybir.AluOpType.is_equal)
                        nc.vector.tensor_tensor(
                            out=S[:, :nv_j, :],
                            in0=S[:, :nv_j, :],
                            in1=dvv_sb[:, v0:v0 + nv_j].unsqueeze(2)
                                .to_broadcast([P, nv_j, P]),
                            op=mybir.AluOpType.mult)
                        if li < 2:
                            pagg = psp.tile([P, P], FP, tag="ps_agg")
                            for i, (v, s) in enumerate(vs):
                                nc.tensor.matmul(
                                    pagg[:], lhsT=strip[:, s, :],
                                    rhs=S[:, v - v0, :],
                                    start=(i == 0), stop=(i == nv_j - 1))
                            if li == 0:
                                # raw-x aggregation: apply W1 post-agg
                                aggsb = bp.tile([P, P], BF, tag="aggsb")
                                nc.scalar.activation(
                                    out=aggsb[:], in_=pagg[:],
                                    func=mybir.ActivationFunctionType.Copy)
                                pz = psp.tile([P, P], FP, tag="ps_z")
                                nc.tensor.matmul(pz[:], lhsT=w1[:],
                                                 rhs=aggsb[:],
                                                 start=True, stop=True)
                                src_ps = pz
                            else:
                                src_ps = pagg
                            nc.scalar.activation(
                                out=Z[:, j * P:(j + 1) * P], in_=src_ps[:],
                                func=mybir.ActivationFunctionType.Copy)
                            nc.vector.bn_stats(
                                out=stats[li][:, j, :],
                                in_=Z[:, j * P:(j + 1) * P])
                        else:
                            pagg = psp.tile([P, n_cls], FP, tag="ps_agg3")
                            for i, (v, s) in enumerate(vs):
                                nc.tensor.matmul(
                                    pagg[:], lhsT=S[:, v - v0, :],
                                    rhs=strip[:, s, :n_cls],
                                    start=(i == 0), stop=(i == nv_j - 1))
                            nc.vector.tensor_add(out=Z3[:, j, :],
                                                 in0=pagg[:], in1=b3bc[:])
                    if li == 2:
                        # log_softmax + output store for this group
                        j0 = pl["blocks"][0]
                        nbl = len(pl["blocks"])
                        zg = Z3[:, j0:j0 + nbl, :]
                        mx = bp.tile([P, GROUP, 1], FP, tag="mx")
                        nc.vector.reduce_max(out=mx[:, :nbl, :], in_=zg,
                                             axis=mybir.AxisListType.X)
                        nc.vector.tensor_sub(
                            out=zg, in0=zg,
                            in1=mx[:, :nbl, :].to_broadcast([P, nbl, n_cls]))
                        exg = bp.tile([P, GROUP, n_cls], FP, tag="exg")
                        nc.scalar.activation(
                            out=exg[:, :nbl, :], in_=zg,
                            func=mybir.ActivationFunctionType.Exp)
                        smg = bp.tile([P, GROUP, 1], FP, tag="smg")
                        nc.vector.reduce_sum(out=smg[:, :nbl, :],
                                             in_=exg[:, :nbl, :],
                                             axis=mybir.AxisListType.X)
                        lsg = bp.tile([P, GROUP, 1], FP, tag="lsg")
                        nc.scalar.activation(
                            out=lsg[:, :nbl, :], in_=smg[:, :nbl, :],
                            func=mybir.ActivationFunctionType.Ln)
                        nc.vector.tensor_sub(
                            out=zg, in0=zg,
                            in1=lsg[:, :nbl, :].to_broadcast([P, nbl, n_cls]))
                        nc.sync.dma_start(out=out_d[:, j0:j0 + nbl, :],
                                          in_=zg)
                    o16_base += o16_len
                    v_base += nv

            # ---------- BN (global) + act
            def bn_relu(g_col, be_col, residual, li, act_out):
                st_out = st_outs[li]
                mv = bp.tile([P, 2], FP, tag="bnmv")
                nc.vector.bn_aggr(out=mv[:], in_=stats[li][:])
                sums = bp.tile([P, 2], FP, tag="sums")
                musq = bp.tile([P, 1], FP, tag="musq")
                nc.vector.tensor_mul(out=musq[:], in0=mv[:, 0:1], in1=mv[:, 0:1])
                nc.scalar.mul(out=sums[:, 0:1], in_=mv[:, 0:1], mul=float(SH))
                nc.vector.tensor_add(out=sums[:, 1:2], in0=mv[:, 1:2],
                                     in1=musq[:])
                nc.scalar.mul(out=sums[:, 1:2], in_=sums[:, 1:2], mul=float(SH))
                nc.sync.dma_start(out=st_in[:], in_=sums[:])
                nc.gpsimd.collective_compute(
                    "AllReduce", mybir.AluOpType.add,
                    replica_groups=[list(range(N_CORES))],
                    ins=[st_in[:]], outs=[st_out[:]])
                gl = bp.tile([P, 2], FP, tag="gl")
                nc.sync.dma_start(out=gl[:], in_=st_out[:])
                mu = bp.tile([P, 1], FP, tag="mu")
                var = bp.tile([P, 1], FP, tag="var")
                nc.scalar.mul(out=mu[:], in_=gl[:, 0:1], mul=1.0 / N)
                nc.scalar.mul(out=var[:], in_=gl[:, 1:2], mul=1.0 / N)
                nc.vector.tensor_mul(out=musq[:], in0=mu[:], in1=mu[:])
                nc.vector.tensor_sub(out=var[:], in0=var[:], in1=musq[:])
                rstd = bp.tile([P, 1], FP, tag="rstd")
                nc.scalar.activation(out=rstd[:], in_=var[:],
                                     func=mybir.ActivationFunctionType.Sqrt,
                                     bias=epsc[:], scale=1.0)
                nc.vector.reciprocal(out=rstd[:], in_=rstd[:])
                sc = bp.tile([P, 1], FP, tag="sc")
                sh = bp.tile([P, 1], FP, tag="sh")
                nc.vector.tensor_mul(out=sc[:], in0=g_col[:], in1=rstd[:])
                nc.vector.tensor_mul(out=sh[:], in0=mu[:], in1=sc[:])
                nc.vector.tensor_sub(out=sh[:], in0=be_col[:], in1=sh[:])
                nc.vector.tensor_scalar(out=Z[:], in0=Z[:], scalar1=sc[:],
                                        scalar2=sh[:],
                                        op0=mybir.AluOpType.mult,
                                        op1=mybir.AluOpType.add)
                if residual:
                    nc.vector.tensor_add(out=Z[:], in0=Z[:], in1=xTbf[:])
                nc.scalar.activation(out=act_out[:], in_=Z[:],
                                     func=mybir.ActivationFunctionType.Relu)

            # ================= pipeline
            layer_agg(0)
            bn_relu(g1c, be1c, residual=True, li=0, act_out=A1)

            table_build(A1, w2, F, 1)
            layer_agg(1)
            bn_relu(g2c, be2c, residual=False, li=1, act_out=A2)

            table_build(A2, w3, n_cls, 2)
            layer_agg(2)          # includes per-group log_softmax + store

    nc.compile()
    return nc


def _make_in_maps(meta, arrs, inputs):
    iota_rows = np.tile(np.arange(P, dtype=np.float32)[None, :], (P, 1))
    shared = dict(
        iotar=iota_rows.astype(ml_dtypes.bfloat16),
        **{k: np.asarray(inputs[k], np.float32)
           for k in ("W1", "W2", "W3", "b3", "g1", "be1", "g2", "be2")})
    return [dict(xshT=arrs["xshT"][c], xsh=arrs["xsh"][c],
                 xtab=arrs["xtab"][c], idx16=arrs["idx16"][c],
                 dlv=arrs["dlv"][c], dvv=arrs["dvv"][c],
                 **shared) for c in range(N_CORES)]


def _unshard(meta, results, n_cls):
    J, SH = meta["J"], meta["SH"]
    out = np.empty((meta["NPAD"], n_cls), np.float32)
    for c in range(N_CORES):
        o = results[c]["out"]                             # [P, J, C]
        nids = ((np.arange(J) * N_CORES + c)[:, None] * P
                + np.arange(P)[None, :])
        out[nids.reshape(-1)] = o.transpose(1, 0, 2).reshape(SH, n_cls)
    full = np.empty((meta["N"], n_cls), np.float32)
    full[meta["order"]] = out[:meta["N"]]
    return full


# ---------------------------------------------------------------- entry

def kernel(x, edge_index, W1, b1, g1, be1, W2, b2, g2, be2, W3, b3):
    x = np.asarray(x, dtype=np.float32)
    edge_index = np.asarray(edge_index)
    N, F = x.shape
    C = np.asarray(W3).shape[1]

    meta, arrs = _host_prep(x, edge_index, N)
    nc = _build(meta, F, C)
    in_maps = _make_in_maps(meta, arrs, dict(
        W1=W1, W2=W2, W3=W3, b3=b3, g1=g1, be1=be1, g2=g2, be2=be2))
    res = run_bass_kernel_spmd(nc, in_maps, core_ids=list(range(N_CORES)))
    return _unshard(meta, res.results, C)
